# revision 11
# baseline (speedup 1.0000x reference)
"""Chamfer distance kernel for Trainium2, 8 NeuronCores, data-parallel over B.

d[i,j] = ||x_i||^2 + ||y_j||^2 - 2<x_i,y_j> realized as a single 5-dim
matmul contraction: z_i = [x_i, 1, ||x_i||^2], w_j = [-2y_j, ||y_j||^2, 1],
d[i,j] = <z_i, w_j>.  Z/W live as replicated 5-row strips at partitions
{0,32,64,96} so four independent matmuls (tile_position row groups) fill a
[128, 2048] PSUM tile (one i-block x j-quarter-chunk) at 4x PE row use.

dist1 (min over j): tensor_reduce(min) straight off PSUM into scr[p,b,q].
dist2 (min over i): in-place tensor_tensor(min) into a persistent SBUF
accumulator, folded across partitions at the end.

The whole loss is reduced to a per-core [1, 2] tensor on device
([sum(dist1), sum(dist2)]), so the host fetch is 8 bytes per core instead
of the 4 MB of dist2 partials the v1 kernel shipped.  Dispatch goes through
a cached jax.jit of the bass_exec custom call so repeat calls skip
retracing/recompiling.
"""

import numpy as np

import concourse.bacc as bacc
import concourse.mybir as mybir
from concourse import masks, tile

F32 = mybir.dt.float32
F32R = mybir.dt.float32r
F16 = mybir.dt.float16
MIN = mybir.AluOpType.min
ADD = mybir.AluOpType.add
MULT = mybir.AluOpType.mult
AXX = mybir.AxisListType.X

B, N, M, D = 8, 8192, 8192, 3
N_CORES = 8
BIG = 3.0e38
BIG16 = 60000.0


def _build_rep(nc, cp, dp, src_dram, n_pts, scale, sq_then_one, tag):
    """Build the [128, n_pts] replicated 5-row matrix for one input cloud.

    Strip rows p0..p0+4 (p0 in {0,32,64,96}): [scale*x0, scale*x1, scale*x2,
    a, b] where (a, b) = (sq, 1) if sq_then_one else (1, sq).
    """
    nt = n_pts // 128
    rep = cp.tile([128, n_pts], F32, tag=f"rep_{tag}")
    xs = cp.tile([128, nt, 3], F32, tag=f"xs_{tag}")
    nc.gpsimd.dma_start(out=xs[:], in_=src_dram.rearrange("(p t) d -> p t d", p=128))
    xsq = cp.tile([128, nt, 3], F32, tag=f"xsq_{tag}")
    nc.vector.tensor_tensor(xsq[:], xs[:], xs[:], op=MULT)
    sq = cp.tile([128, nt], F32, tag=f"sq_{tag}")
    nc.vector.tensor_reduce(sq[:], xsq[:], axis=AXX, op=ADD)
    sq_d = dp.tile([n_pts], F32, tag=f"sqd_{tag}")
    nc.gpsimd.dma_start(out=sq_d.rearrange("(p t) -> p t", p=128), in_=sq[:])
    xt = cp.tile([128, 3, nt], F32, tag=f"xt_{tag}")
    nc.vector.tensor_scalar_mul(xt.rearrange("p d t -> p t d"), xs[:], scale)
    xt_d = dp.tile([3, n_pts], F32, tag=f"xtd_{tag}")
    nc.gpsimd.dma_start(out=xt_d.rearrange("d (p t) -> p d t", p=128), in_=xt[:])
    ones = cp.tile([1, n_pts], F32, tag=f"ones_{tag}")
    nc.vector.memset(ones[:], 1.0)
    sq_row = sq_d.rearrange("(a q) -> a q", a=1)
    for r in range(4):
        p0 = 32 * r
        nc.gpsimd.dma_start(out=rep[p0 : p0 + 3, :], in_=xt_d[:])
        if sq_then_one:
            nc.gpsimd.dma_start(out=rep[p0 + 3 : p0 + 4, :], in_=sq_row)
            nc.gpsimd.dma_start(out=rep[p0 + 4 : p0 + 5, :], in_=ones[:])
        else:
            nc.gpsimd.dma_start(out=rep[p0 + 3 : p0 + 4, :], in_=ones[:])
            nc.gpsimd.dma_start(out=rep[p0 + 4 : p0 + 5, :], in_=sq_row)
    return rep


def build_chamfer_nc(n=N, m=M, n_cores=N_CORES, mm_f32r=False, min16=True, iters=1):
    nc = bacc.Bacc("TRN2", num_devices=n_cores)
    x_d = nc.dram_tensor("input1", [n, 3], F32, kind="ExternalInput")
    y_d = nc.dram_tensor("input2", [m, 3], F32, kind="ExternalInput")
    n_blk = n // 128
    chunk = min(2048, m)
    n_chunks = m // chunk
    strip_w = min(512, chunk)
    n_strips = chunk // strip_w
    out_d = nc.dram_tensor("out", [1, 2], F32, kind="ExternalOutput")
    acc_dt = F16 if min16 else F32
    acc_big = BIG16 if min16 else BIG

    with tile.TileContext(nc) as tc:
        with (
            tc.tile_pool(name="c", bufs=1) as cp,
            tc.tile_pool(name="sc", bufs=3) as sp,
            tc.tile_pool(name="cv", bufs=3) as vp,
            tc.tile_pool(name="dr", bufs=1, space="DRAM") as dp,
        ):
            # z side from input1 (rows [x,1,sq]); w side from input2 ([-2y,sq,1])
            zrep = _build_rep(nc, cp, dp, x_d, n, 1.0, False, "z")
            wrep = _build_rep(nc, cp, dp, y_d, m, -2.0, True, "w")

            acc = cp.tile([128, m], acc_dt, tag="acc")
            nc.vector.memset(acc[:], acc_big)
            scr = cp.tile([128, n_blk * n_chunks], acc_dt, tag="scr")

            with tc.tile_pool(name="ps", bufs=2, space="PSUM") as pp:
                for it in range(iters):
                    for b in range(n_blk):
                        i0 = b * 128
                        for q in range(n_chunks):
                            j0 = q * chunk
                            idx = b * n_chunks + q
                            ps = pp.tile([128, chunk], F32, tag="ps")
                            for s in range(n_strips):
                                p0 = 32 * (s % 4)
                                lhsT = zrep[p0 : p0 + 5, i0 : i0 + 128]
                                rhs = wrep[
                                    p0 : p0 + 5,
                                    j0 + s * strip_w : j0 + (s + 1) * strip_w,
                                ]
                                if mm_f32r:
                                    lhsT = lhsT.bitcast(F32R)
                                    rhs = rhs.bitcast(F32R)
                                nc.tensor.matmul(
                                    ps[:, s * strip_w : (s + 1) * strip_w],
                                    lhsT=lhsT,
                                    rhs=rhs,
                                    tile_position=(p0, 0),
                                )
                            if min16:
                                src = vp.tile([128, chunk], F16, tag="cv")
                                nc.scalar.copy(src[:], ps[:])
                            else:
                                src = ps
                            nc.vector.tensor_reduce(
                                scr[:, idx : idx + 1], src[:], axis=AXX, op=MIN
                            )
                            nc.vector.tensor_tensor(
                                acc[:, j0 : j0 + chunk],
                                acc[:, j0 : j0 + chunk],
                                src[:],
                                op=MIN,
                            )

            if min16:
                accf = cp.tile([128, m], F32, tag="accf")
                nc.scalar.copy(accf[:], acc[:])
                scrf = sp.tile([128, n_blk * n_chunks], F32, tag="scrf")
                nc.scalar.copy(scrf[:], scr[:])
            else:
                accf = acc
                scrf = scr

            # dist1: min over chunks, then sum over blocks
            d1min = sp.tile([128, n_blk], F32, tag="d1min")
            nc.vector.tensor_reduce(
                d1min[:],
                scrf[:].rearrange("p (b q) -> p b q", b=n_blk),
                axis=AXX,
                op=MIN,
            )
            s1 = sp.tile([128, 1], F32, tag="s1")
            nc.vector.tensor_reduce(s1[:], d1min[:], axis=AXX, op=ADD)

            # dist2: per-128-block PE transpose, rowmin, then sum
            ident = cp.tile([128, 128], F32, tag="ident")
            masks.make_identity(nc, ident[:])
            d2col = cp.tile([128, n_blk], F32, tag="d2col")
            with tc.tile_pool(name="pt", bufs=2, space="PSUM") as pt:
                for t in range(n_blk):
                    pst = pt.tile([128, 128], F32, tag="pst")
                    nc.tensor.transpose(
                        pst[:], accf[:, t * 128 : (t + 1) * 128], ident[:]
                    )
                    nc.vector.tensor_reduce(
                        d2col[:, t : t + 1], pst[:], axis=AXX, op=MIN
                    )
            s2 = sp.tile([128, 1], F32, tag="s2")
            nc.vector.tensor_reduce(s2[:], d2col[:], axis=AXX, op=ADD)

            # partition sums via ones-matmul: out[0, c] = sum_p s_c[p]
            onesc = sp.tile([128, 1], F32, tag="onesc")
            nc.vector.memset(onesc[:], 1.0)
            outsb = sp.tile([1, 2], F32, tag="outsb")
            with tc.tile_pool(name="pf", bufs=1, space="PSUM") as pf:
                pssum = pf.tile([1, 2], F32, tag="pssum")
                nc.tensor.matmul(pssum[0:1, 0:1], lhsT=s1[:], rhs=onesc[:])
                nc.tensor.matmul(pssum[0:1, 1:2], lhsT=s2[:], rhs=onesc[:])
                nc.scalar.copy(outsb[:], pssum[:])
            nc.gpsimd.dma_start(out=out_d[:], in_=outsb[:])

    nc.compile()
    return nc


_RUNNER = None


def _build_runner():
    import jax
    from jax.sharding import Mesh, PartitionSpec

    from jax.experimental.shard_map import shard_map
    from concourse import bass2jax as b2j

    nc = build_chamfer_nc()
    b2j.install_neuronx_cc_hook()
    partition_name = nc.partition_id_tensor.name if nc.partition_id_tensor else None
    in_names, out_names, out_avals, zero_shapes = [], [], [], []
    for alloc in nc.m.functions[0].allocations:
        if not isinstance(alloc, mybir.MemoryLocationSet):
            continue
        name = alloc.memorylocations[0].name
        if alloc.kind == "ExternalInput":
            if name != partition_name:
                in_names.append(name)
        elif alloc.kind == "ExternalOutput":
            out_names.append(name)
            shape = tuple(alloc.tensor_shape)
            dtype = mybir.dt.np(alloc.dtype)
            out_avals.append(jax.core.ShapedArray(shape, dtype))
            zero_shapes.append((shape, dtype))
    n_params = len(in_names)
    n_outs = len(out_avals)
    all_in_names = list(in_names) + list(out_names)
    if partition_name is not None:
        all_in_names.append(partition_name)
    donate = tuple(range(n_params, n_params + n_outs))

    def _body(*args):
        operands = list(args)
        if partition_name is not None:
            operands.append(b2j.partition_id_tensor())
        outs = b2j._bass_exec_p.bind(
            *operands,
            out_avals=tuple(out_avals),
            in_names=tuple(all_in_names),
            out_names=tuple(out_names),
            lowering_input_output_aliases=(),
            sim_require_finite=True,
            sim_require_nnan=True,
            nc=nc,
        )
        return tuple(outs)

    devices = jax.devices()[:N_CORES]
    mesh = Mesh(np.asarray(devices), ("core",))
    in_specs = (PartitionSpec("core"),) * (n_params + n_outs)
    out_specs = (PartitionSpec("core"),) * len(out_names)
    sharded = jax.jit(
        shard_map(
            _body, mesh=mesh, in_specs=in_specs, out_specs=out_specs, check_rep=False
        ),
        donate_argnums=donate,
        keep_unused=True,
    )
    return sharded, in_names, out_names, out_avals, zero_shapes


def kernel(input1: np.ndarray, input2: np.ndarray) -> np.ndarray:
    global _RUNNER
    input1 = np.ascontiguousarray(np.asarray(input1, dtype=np.float32))
    input2 = np.ascontiguousarray(np.asarray(input2, dtype=np.float32))
    bsz, n, _ = input1.shape
    m = input2.shape[1]
    if _RUNNER is None:
        _RUNNER = _build_runner()
    sharded, in_names, out_names, out_avals, zero_shapes = _RUNNER
    by_name = {
        "input1": input1.reshape(bsz * n, 3),
        "input2": input2.reshape(bsz * m, 3),
    }
    concat_in = [by_name[name] for name in in_names]
    concat_zeros = [
        np.zeros((N_CORES * shape[0], *shape[1:]), dtype) for shape, dtype in zero_shapes
    ]
    out_arrs = sharded(*concat_in, *concat_zeros)
    out = np.asarray(out_arrs[out_names.index("out")]).reshape(N_CORES, 2)
    s = out.astype(np.float64).sum(axis=0)
    loss = s[0] / (bsz * n) + s[1] / (bsz * m)
    return np.float32(loss)


# revision 12
# speedup vs baseline: 1.4485x; 1.4485x over previous
"""Chamfer distance kernel for Trainium2, 8 NeuronCores, data-parallel over B.

d[i,j] = ||x_i||^2 + ||y_j||^2 - 2<x_i,y_j> realized as a single 5-dim
matmul contraction: z_i = [x_i, 1, ||x_i||^2], w_j = [-2y_j, ||y_j||^2, 1],
d[i,j] = <z_i, w_j>.  Z/W live as replicated 5-row strips at partitions
{0,32,64,96} so four independent matmuls (tile_position row groups) fill a
[128, 2048] PSUM tile (one i-block x j-quarter-chunk) at 4x PE row use.

The matmul stays fp32 (d is a catastrophic cancellation of O(10) terms down
to ~1e-3 mins; fp32r/bf16 inputs corrupt it), but each PSUM tile is then
converted once by the Scalar engine to an fp16 SBUF copy — post-matmul the
error is relative to d itself, so fp16 is safe — and both min passes run on
the DVE at 16-bit packed throughput:
  dist1 (min over j): tensor_reduce(min) into scr[p, b*q].
  dist2 (min over i): in-place tensor_tensor(min) into a persistent fp16
  accumulator; partition mins via PE transpose + rowmin at the end.

The whole loss is reduced to a per-core [1, 2] tensor on device
([sum(dist1), sum(dist2)]; partition sums via ones-matmul), so the host
fetch is 8 bytes per core instead of the 4 MB of dist2 partials the v1
kernel shipped.  Dispatch goes through a cached jax.jit of the bass_exec
custom call so repeat calls skip retracing/recompiling — the warm call is
a single axon round trip (~65-90 ms, infrastructure floor; input upload
and the ~0.6 ms device execution hide under it).
"""

import numpy as np

import concourse.bacc as bacc
import concourse.mybir as mybir
from concourse import masks, tile

F32 = mybir.dt.float32
F32R = mybir.dt.float32r
F16 = mybir.dt.float16
MIN = mybir.AluOpType.min
ADD = mybir.AluOpType.add
MULT = mybir.AluOpType.mult
AXX = mybir.AxisListType.X

B, N, M, D = 8, 8192, 8192, 3
N_CORES = 8
BIG = 3.0e38
BIG16 = 60000.0


def _build_rep(nc, cp, dp, src_dram, n_pts, scale, sq_then_one, tag):
    """Build the [128, n_pts] replicated 5-row matrix for one input cloud.

    Strip rows p0..p0+4 (p0 in {0,32,64,96}): [scale*x0, scale*x1, scale*x2,
    a, b] where (a, b) = (sq, 1) if sq_then_one else (1, sq).
    """
    nt = n_pts // 128
    rep = cp.tile([128, n_pts], F32, tag=f"rep_{tag}")
    xs = cp.tile([128, nt, 3], F32, tag=f"xs_{tag}")
    nc.gpsimd.dma_start(out=xs[:], in_=src_dram.rearrange("(p t) d -> p t d", p=128))
    xsq = cp.tile([128, nt, 3], F32, tag=f"xsq_{tag}")
    nc.vector.tensor_tensor(xsq[:], xs[:], xs[:], op=MULT)
    sq = cp.tile([128, nt], F32, tag=f"sq_{tag}")
    nc.vector.tensor_reduce(sq[:], xsq[:], axis=AXX, op=ADD)
    sq_d = dp.tile([n_pts], F32, tag=f"sqd_{tag}")
    nc.gpsimd.dma_start(out=sq_d.rearrange("(p t) -> p t", p=128), in_=sq[:])
    xt = cp.tile([128, 3, nt], F32, tag=f"xt_{tag}")
    nc.vector.tensor_scalar_mul(xt.rearrange("p d t -> p t d"), xs[:], scale)
    xt_d = dp.tile([3, n_pts], F32, tag=f"xtd_{tag}")
    nc.gpsimd.dma_start(out=xt_d.rearrange("d (p t) -> p d t", p=128), in_=xt[:])
    ones = cp.tile([1, n_pts], F32, tag=f"ones_{tag}")
    nc.vector.memset(ones[:], 1.0)
    sq_row = sq_d.rearrange("(a q) -> a q", a=1)
    for r in range(4):
        p0 = 32 * r
        nc.gpsimd.dma_start(out=rep[p0 : p0 + 3, :], in_=xt_d[:])
        if sq_then_one:
            nc.gpsimd.dma_start(out=rep[p0 + 3 : p0 + 4, :], in_=sq_row)
            nc.gpsimd.dma_start(out=rep[p0 + 4 : p0 + 5, :], in_=ones[:])
        else:
            nc.gpsimd.dma_start(out=rep[p0 + 3 : p0 + 4, :], in_=ones[:])
            nc.gpsimd.dma_start(out=rep[p0 + 4 : p0 + 5, :], in_=sq_row)
    return rep


def build_chamfer_nc(n=N, m=M, n_cores=N_CORES, mm_f32r=False, min16=True, iters=1):
    nc = bacc.Bacc("TRN2", num_devices=n_cores)
    x_d = nc.dram_tensor("input1", [n, 3], F32, kind="ExternalInput")
    y_d = nc.dram_tensor("input2", [m, 3], F32, kind="ExternalInput")
    n_blk = n // 128
    chunk = min(2048, m)
    n_chunks = m // chunk
    strip_w = min(512, chunk)
    n_strips = chunk // strip_w
    out_d = nc.dram_tensor("out", [1, 2], F32, kind="ExternalOutput")
    acc_dt = F16 if min16 else F32
    acc_big = BIG16 if min16 else BIG

    with tile.TileContext(nc) as tc:
        with (
            tc.tile_pool(name="c", bufs=1) as cp,
            tc.tile_pool(name="sc", bufs=3) as sp,
            tc.tile_pool(name="cv", bufs=3) as vp,
            tc.tile_pool(name="dr", bufs=1, space="DRAM") as dp,
        ):
            # z side from input1 (rows [x,1,sq]); w side from input2 ([-2y,sq,1])
            zrep = _build_rep(nc, cp, dp, x_d, n, 1.0, False, "z")
            wrep = _build_rep(nc, cp, dp, y_d, m, -2.0, True, "w")

            acc = cp.tile([128, m], acc_dt, tag="acc")
            nc.vector.memset(acc[:], acc_big)
            scr = cp.tile([128, n_blk * n_chunks], acc_dt, tag="scr")

            with tc.tile_pool(name="ps", bufs=2, space="PSUM") as pp:
                for it in range(iters):
                    for b in range(n_blk):
                        i0 = b * 128
                        for q in range(n_chunks):
                            j0 = q * chunk
                            idx = b * n_chunks + q
                            ps = pp.tile([128, chunk], F32, tag="ps")
                            for s in range(n_strips):
                                p0 = 32 * (s % 4)
                                lhsT = zrep[p0 : p0 + 5, i0 : i0 + 128]
                                rhs = wrep[
                                    p0 : p0 + 5,
                                    j0 + s * strip_w : j0 + (s + 1) * strip_w,
                                ]
                                if mm_f32r:
                                    lhsT = lhsT.bitcast(F32R)
                                    rhs = rhs.bitcast(F32R)
                                nc.tensor.matmul(
                                    ps[:, s * strip_w : (s + 1) * strip_w],
                                    lhsT=lhsT,
                                    rhs=rhs,
                                    tile_position=(p0, 0),
                                )
                            if min16:
                                src = vp.tile([128, chunk], F16, tag="cv")
                                nc.scalar.copy(src[:], ps[:])
                            else:
                                src = ps
                            nc.vector.tensor_reduce(
                                scr[:, idx : idx + 1], src[:], axis=AXX, op=MIN
                            )
                            nc.vector.tensor_tensor(
                                acc[:, j0 : j0 + chunk],
                                acc[:, j0 : j0 + chunk],
                                src[:],
                                op=MIN,
                            )

            if min16:
                accf = cp.tile([128, m], F32, tag="accf")
                nc.scalar.copy(accf[:], acc[:])
                scrf = sp.tile([128, n_blk * n_chunks], F32, tag="scrf")
                nc.scalar.copy(scrf[:], scr[:])
            else:
                accf = acc
                scrf = scr

            # dist1: min over chunks, then sum over blocks
            d1min = sp.tile([128, n_blk], F32, tag="d1min")
            nc.vector.tensor_reduce(
                d1min[:],
                scrf[:].rearrange("p (b q) -> p b q", b=n_blk),
                axis=AXX,
                op=MIN,
            )
            s1 = sp.tile([128, 1], F32, tag="s1")
            nc.vector.tensor_reduce(s1[:], d1min[:], axis=AXX, op=ADD)

            # dist2: per-128-block PE transpose, rowmin, then sum
            ident = cp.tile([128, 128], F32, tag="ident")
            masks.make_identity(nc, ident[:])
            d2col = cp.tile([128, n_blk], F32, tag="d2col")
            with tc.tile_pool(name="pt", bufs=2, space="PSUM") as pt:
                for t in range(n_blk):
                    pst = pt.tile([128, 128], F32, tag="pst")
                    nc.tensor.transpose(
                        pst[:], accf[:, t * 128 : (t + 1) * 128], ident[:]
                    )
                    nc.vector.tensor_reduce(
                        d2col[:, t : t + 1], pst[:], axis=AXX, op=MIN
                    )
            s2 = sp.tile([128, 1], F32, tag="s2")
            nc.vector.tensor_reduce(s2[:], d2col[:], axis=AXX, op=ADD)

            # partition sums via ones-matmul: out[0, c] = sum_p s_c[p]
            onesc = sp.tile([128, 1], F32, tag="onesc")
            nc.vector.memset(onesc[:], 1.0)
            outsb = sp.tile([1, 2], F32, tag="outsb")
            with tc.tile_pool(name="pf", bufs=1, space="PSUM") as pf:
                pssum = pf.tile([1, 2], F32, tag="pssum")
                nc.tensor.matmul(pssum[0:1, 0:1], lhsT=s1[:], rhs=onesc[:])
                nc.tensor.matmul(pssum[0:1, 1:2], lhsT=s2[:], rhs=onesc[:])
                nc.scalar.copy(outsb[:], pssum[:])
            nc.gpsimd.dma_start(out=out_d[:], in_=outsb[:])

    nc.compile()
    return nc


_RUNNER = None


def _build_runner():
    import jax
    from jax.sharding import Mesh, PartitionSpec

    from jax.experimental.shard_map import shard_map
    from concourse import bass2jax as b2j

    nc = build_chamfer_nc()
    b2j.install_neuronx_cc_hook()
    partition_name = nc.partition_id_tensor.name if nc.partition_id_tensor else None
    in_names, out_names, out_avals, zero_shapes = [], [], [], []
    for alloc in nc.m.functions[0].allocations:
        if not isinstance(alloc, mybir.MemoryLocationSet):
            continue
        name = alloc.memorylocations[0].name
        if alloc.kind == "ExternalInput":
            if name != partition_name:
                in_names.append(name)
        elif alloc.kind == "ExternalOutput":
            out_names.append(name)
            shape = tuple(alloc.tensor_shape)
            dtype = mybir.dt.np(alloc.dtype)
            out_avals.append(jax.core.ShapedArray(shape, dtype))
            zero_shapes.append((shape, dtype))
    n_params = len(in_names)
    n_outs = len(out_avals)
    all_in_names = list(in_names) + list(out_names)
    if partition_name is not None:
        all_in_names.append(partition_name)
    donate = tuple(range(n_params, n_params + n_outs))

    def _body(*args):
        operands = list(args)
        if partition_name is not None:
            operands.append(b2j.partition_id_tensor())
        outs = b2j._bass_exec_p.bind(
            *operands,
            out_avals=tuple(out_avals),
            in_names=tuple(all_in_names),
            out_names=tuple(out_names),
            lowering_input_output_aliases=(),
            sim_require_finite=True,
            sim_require_nnan=True,
            nc=nc,
        )
        return tuple(outs)

    devices = jax.devices()[:N_CORES]
    mesh = Mesh(np.asarray(devices), ("core",))
    in_specs = (PartitionSpec("core"),) * (n_params + n_outs)
    out_specs = (PartitionSpec("core"),) * len(out_names)
    sharded = jax.jit(
        shard_map(
            _body, mesh=mesh, in_specs=in_specs, out_specs=out_specs, check_rep=False
        ),
        donate_argnums=donate,
        keep_unused=True,
    )
    return sharded, in_names, out_names, out_avals, zero_shapes


def kernel(input1: np.ndarray, input2: np.ndarray) -> np.ndarray:
    global _RUNNER
    input1 = np.ascontiguousarray(np.asarray(input1, dtype=np.float32))
    input2 = np.ascontiguousarray(np.asarray(input2, dtype=np.float32))
    bsz, n, _ = input1.shape
    m = input2.shape[1]
    if _RUNNER is None:
        _RUNNER = _build_runner()
    sharded, in_names, out_names, out_avals, zero_shapes = _RUNNER
    by_name = {
        "input1": input1.reshape(bsz * n, 3),
        "input2": input2.reshape(bsz * m, 3),
    }
    concat_in = [by_name[name] for name in in_names]
    concat_zeros = [
        np.zeros((N_CORES * shape[0], *shape[1:]), dtype) for shape, dtype in zero_shapes
    ]
    out_arrs = sharded(*concat_in, *concat_zeros)
    out = np.asarray(out_arrs[out_names.index("out")]).reshape(N_CORES, 2)
    s = out.astype(np.float64).sum(axis=0)
    loss = s[0] / (bsz * n) + s[1] / (bsz * m)
    return np.float32(loss)


# revision 17
# speedup vs baseline: 1.5165x; 1.0469x over previous
"""Chamfer distance kernel for Trainium2, 8 NeuronCores, data-parallel over B.

d[i,j] = ||x_i||^2 + ||y_j||^2 - 2<x_i,y_j> realized as a single 5-dim
matmul contraction: z_i = [x_i, 1, ||x_i||^2], w_j = [-2y_j, ||y_j||^2, 1],
d[i,j] = <z_i, w_j>.  Z/W live as replicated 5-row strips at partitions
{0,32,64,96} so four independent matmuls (tile_position row groups) fill a
[128, 2048] PSUM tile (one i-block x j-quarter-chunk) at 4x PE row use.

The matmul stays fp32 (d is a catastrophic cancellation of O(10) terms down
to ~1e-3 mins; fp32r/bf16 inputs corrupt it), but each PSUM tile is then
converted once by the Scalar engine to an fp16 SBUF copy — post-matmul the
error is relative to d itself, so fp16 is safe — and both min passes run on
the DVE at 16-bit packed throughput:
  dist1 (min over j): tensor_reduce(min) into scr[p, b*q].
  dist2 (min over i): in-place tensor_tensor(min) into a persistent fp16
  accumulator; partition mins via PE transpose + rowmin at the end.

The whole loss is reduced to a per-core [1, 2] tensor on device
([sum(dist1), sum(dist2)]; partition sums via ones-matmul), so the host
fetch is 8 bytes per core instead of the 4 MB of dist2 partials the v1
kernel shipped.  Dispatch goes through a cached jax.jit of the bass_exec
custom call so repeat calls skip retracing/recompiling — the warm call is
a single axon round trip (~65-90 ms, infrastructure floor; input upload
and the ~0.6 ms device execution hide under it).
"""

import numpy as np

import concourse.bacc as bacc
import concourse.mybir as mybir
from concourse import masks, tile

F32 = mybir.dt.float32
F32R = mybir.dt.float32r
F16 = mybir.dt.float16
MIN = mybir.AluOpType.min
ADD = mybir.AluOpType.add
MULT = mybir.AluOpType.mult
AXX = mybir.AxisListType.X

B, N, M, D = 8, 8192, 8192, 3
N_CORES = 8
BIG = 3.0e38
BIG16 = 60000.0


def _build_rep(nc, cp, dp, src_dram, n_pts, scale, sq_then_one, tag):
    """Build the [128, n_pts] replicated 5-row matrix for one input cloud.

    Strip rows p0..p0+4 (p0 in {0,32,64,96}): [scale*x0, scale*x1, scale*x2,
    a, b] where (a, b) = (sq, 1) if sq_then_one else (1, sq).
    """
    nt = n_pts // 128
    rep = cp.tile([128, n_pts], F32, tag=f"rep_{tag}")
    xs = cp.tile([128, nt, 3], F32, tag=f"xs_{tag}")
    nc.gpsimd.dma_start(out=xs[:], in_=src_dram.rearrange("(p t) d -> p t d", p=128))
    xsq = cp.tile([128, nt, 3], F32, tag=f"xsq_{tag}")
    nc.vector.tensor_tensor(xsq[:], xs[:], xs[:], op=MULT)
    sq = cp.tile([128, nt], F32, tag=f"sq_{tag}")
    nc.vector.tensor_reduce(sq[:], xsq[:], axis=AXX, op=ADD)
    sq_d = dp.tile([n_pts], F32, tag=f"sqd_{tag}")
    nc.gpsimd.dma_start(out=sq_d.rearrange("(p t) -> p t", p=128), in_=sq[:])
    xt = cp.tile([128, 3, nt], F32, tag=f"xt_{tag}")
    nc.vector.tensor_scalar_mul(xt.rearrange("p d t -> p t d"), xs[:], scale)
    xt_d = dp.tile([3, n_pts], F32, tag=f"xtd_{tag}")
    nc.gpsimd.dma_start(out=xt_d.rearrange("d (p t) -> p d t", p=128), in_=xt[:])
    ones = cp.tile([1, n_pts], F32, tag=f"ones_{tag}")
    nc.vector.memset(ones[:], 1.0)
    sq_row = sq_d.rearrange("(a q) -> a q", a=1)
    for r in range(4):
        p0 = 32 * r
        nc.gpsimd.dma_start(out=rep[p0 : p0 + 3, :], in_=xt_d[:])
        if sq_then_one:
            nc.gpsimd.dma_start(out=rep[p0 + 3 : p0 + 4, :], in_=sq_row)
            nc.gpsimd.dma_start(out=rep[p0 + 4 : p0 + 5, :], in_=ones[:])
        else:
            nc.gpsimd.dma_start(out=rep[p0 + 3 : p0 + 4, :], in_=ones[:])
            nc.gpsimd.dma_start(out=rep[p0 + 4 : p0 + 5, :], in_=sq_row)
    return rep


def build_chamfer_nc(
    n=N, m=M, n_cores=N_CORES, mm_f32r=False, min16=True, iters=1, fuse=False, pool_tt=False
):
    nc = bacc.Bacc("TRN2", num_devices=n_cores)
    x_d = nc.dram_tensor("input1", [n, 3], F32, kind="ExternalInput")
    y_d = nc.dram_tensor("input2", [m, 3], F32, kind="ExternalInput")
    n_blk = n // 128
    chunk = min(2048, m)
    n_chunks = m // chunk
    strip_w = min(512, chunk)
    n_strips = chunk // strip_w
    out_d = nc.dram_tensor("out", [1, 2], F32, kind="ExternalOutput")
    acc_dt = F16 if min16 else F32
    acc_big = BIG16 if min16 else BIG

    with tile.TileContext(nc) as tc:
        with (
            tc.tile_pool(name="c", bufs=1) as cp,
            tc.tile_pool(name="sc", bufs=3) as sp,
            tc.tile_pool(name="cv", bufs=3) as vp,
            tc.tile_pool(name="dr", bufs=1, space="DRAM") as dp,
        ):
            # z side from input1 (rows [x,1,sq]); w side from input2 ([-2y,sq,1])
            zrep = _build_rep(nc, cp, dp, x_d, n, 1.0, False, "z")
            wrep = _build_rep(nc, cp, dp, y_d, m, -2.0, True, "w")

            acc = cp.tile([128, m], acc_dt, tag="acc")
            nc.vector.memset(acc[:], acc_big)
            scr = cp.tile([128, n_blk * n_chunks], acc_dt, tag="scr")
            junk = None
            if min16 and fuse:
                junk = cp.tile([128, chunk], F16, tag="junk")

            with tc.tile_pool(name="ps", bufs=2, space="PSUM") as pp:
                for it in range(iters):
                    for b in range(n_blk):
                        i0 = b * 128
                        for q in range(n_chunks):
                            j0 = q * chunk
                            idx = b * n_chunks + q
                            ps = pp.tile([128, chunk], F32, tag="ps")
                            for s in range(n_strips):
                                p0 = 32 * (s % 4)
                                lhsT = zrep[p0 : p0 + 5, i0 : i0 + 128]
                                rhs = wrep[
                                    p0 : p0 + 5,
                                    j0 + s * strip_w : j0 + (s + 1) * strip_w,
                                ]
                                if mm_f32r:
                                    lhsT = lhsT.bitcast(F32R)
                                    rhs = rhs.bitcast(F32R)
                                nc.tensor.matmul(
                                    ps[:, s * strip_w : (s + 1) * strip_w],
                                    lhsT=lhsT,
                                    rhs=rhs,
                                    tile_position=(p0, 0),
                                )
                            if min16:
                                src = vp.tile([128, chunk], F16, tag="cv")
                                nc.scalar.copy(src[:], ps[:])
                            else:
                                src = ps
                            if min16 and fuse:
                                # rowmin fused onto a 2x-packed TT pass; the
                                # full-tile `out` is a don't-care scratch
                                nc.vector.tensor_tensor_reduce(
                                    out=junk[:],
                                    in0=src[:],
                                    in1=src[:],
                                    scale=1.0,
                                    scalar=float(BIG16),
                                    op0=MIN,
                                    op1=MIN,
                                    accum_out=scr[:, idx : idx + 1],
                                )
                            else:
                                nc.vector.tensor_reduce(
                                    scr[:, idx : idx + 1], src[:], axis=AXX, op=MIN
                                )
                            tt_eng = (
                                nc.gpsimd
                                if (min16 and pool_tt and idx % 2 == 1)
                                else nc.vector
                            )
                            tt_eng.tensor_tensor(
                                acc[:, j0 : j0 + chunk],
                                acc[:, j0 : j0 + chunk],
                                src[:],
                                op=MIN,
                            )

            if min16:
                accf = cp.tile([128, m], F32, tag="accf")
                nc.scalar.copy(accf[:], acc[:])
                scrf = sp.tile([128, n_blk * n_chunks], F32, tag="scrf")
                nc.scalar.copy(scrf[:], scr[:])
            else:
                accf = acc
                scrf = scr

            # dist1: min over chunks, then sum over blocks
            d1min = sp.tile([128, n_blk], F32, tag="d1min")
            nc.vector.tensor_reduce(
                d1min[:],
                scrf[:].rearrange("p (b q) -> p b q", b=n_blk),
                axis=AXX,
                op=MIN,
            )
            s1 = sp.tile([128, 1], F32, tag="s1")
            nc.vector.tensor_reduce(s1[:], d1min[:], axis=AXX, op=ADD)

            # dist2: per-128-block PE transpose, rowmin, then sum
            ident = cp.tile([128, 128], F32, tag="ident")
            masks.make_identity(nc, ident[:])
            d2col = cp.tile([128, n_blk], F32, tag="d2col")
            with tc.tile_pool(name="pt", bufs=2, space="PSUM") as pt:
                for t in range(n_blk):
                    pst = pt.tile([128, 128], F32, tag="pst")
                    nc.tensor.transpose(
                        pst[:], accf[:, t * 128 : (t + 1) * 128], ident[:]
                    )
                    nc.vector.tensor_reduce(
                        d2col[:, t : t + 1], pst[:], axis=AXX, op=MIN
                    )
            s2 = sp.tile([128, 1], F32, tag="s2")
            nc.vector.tensor_reduce(s2[:], d2col[:], axis=AXX, op=ADD)

            # partition sums via ones-matmul: out[0, c] = sum_p s_c[p]
            onesc = sp.tile([128, 1], F32, tag="onesc")
            nc.vector.memset(onesc[:], 1.0)
            outsb = sp.tile([1, 2], F32, tag="outsb")
            with tc.tile_pool(name="pf", bufs=1, space="PSUM") as pf:
                pssum = pf.tile([1, 2], F32, tag="pssum")
                nc.tensor.matmul(pssum[0:1, 0:1], lhsT=s1[:], rhs=onesc[:])
                nc.tensor.matmul(pssum[0:1, 1:2], lhsT=s2[:], rhs=onesc[:])
                nc.scalar.copy(outsb[:], pssum[:])
            nc.gpsimd.dma_start(out=out_d[:], in_=outsb[:])

    nc.compile()
    return nc


_RUNNER = None


def _build_runner():
    import jax
    from jax.sharding import Mesh, PartitionSpec

    from jax.experimental.shard_map import shard_map
    from concourse import bass2jax as b2j

    nc = build_chamfer_nc()
    b2j.install_neuronx_cc_hook()
    partition_name = nc.partition_id_tensor.name if nc.partition_id_tensor else None
    in_names, out_names, out_avals, zero_shapes = [], [], [], []
    for alloc in nc.m.functions[0].allocations:
        if not isinstance(alloc, mybir.MemoryLocationSet):
            continue
        name = alloc.memorylocations[0].name
        if alloc.kind == "ExternalInput":
            if name != partition_name:
                in_names.append(name)
        elif alloc.kind == "ExternalOutput":
            out_names.append(name)
            shape = tuple(alloc.tensor_shape)
            dtype = mybir.dt.np(alloc.dtype)
            out_avals.append(jax.core.ShapedArray(shape, dtype))
            zero_shapes.append((shape, dtype))
    n_params = len(in_names)
    n_outs = len(out_avals)
    all_in_names = list(in_names) + list(out_names)
    if partition_name is not None:
        all_in_names.append(partition_name)
    donate = tuple(range(n_params, n_params + n_outs))

    def _body(*args):
        operands = list(args)
        if partition_name is not None:
            operands.append(b2j.partition_id_tensor())
        outs = b2j._bass_exec_p.bind(
            *operands,
            out_avals=tuple(out_avals),
            in_names=tuple(all_in_names),
            out_names=tuple(out_names),
            lowering_input_output_aliases=(),
            sim_require_finite=True,
            sim_require_nnan=True,
            nc=nc,
        )
        return tuple(outs)

    devices = jax.devices()[:N_CORES]
    mesh = Mesh(np.asarray(devices), ("core",))
    in_specs = (PartitionSpec("core"),) * (n_params + n_outs)
    out_specs = (PartitionSpec("core"),) * len(out_names)
    sharded = jax.jit(
        shard_map(
            _body, mesh=mesh, in_specs=in_specs, out_specs=out_specs, check_rep=False
        ),
        donate_argnums=donate,
        keep_unused=True,
    )
    return sharded, in_names, out_names, out_avals, zero_shapes


def kernel(input1: np.ndarray, input2: np.ndarray) -> np.ndarray:
    global _RUNNER
    input1 = np.ascontiguousarray(np.asarray(input1, dtype=np.float32))
    input2 = np.ascontiguousarray(np.asarray(input2, dtype=np.float32))
    bsz, n, _ = input1.shape
    m = input2.shape[1]
    if _RUNNER is None:
        _RUNNER = _build_runner()
    sharded, in_names, out_names, out_avals, zero_shapes = _RUNNER
    by_name = {
        "input1": input1.reshape(bsz * n, 3),
        "input2": input2.reshape(bsz * m, 3),
    }
    concat_in = [by_name[name] for name in in_names]
    concat_zeros = [
        np.zeros((N_CORES * shape[0], *shape[1:]), dtype) for shape, dtype in zero_shapes
    ]
    out_arrs = sharded(*concat_in, *concat_zeros)
    out = np.asarray(out_arrs[out_names.index("out")]).reshape(N_CORES, 2)
    s = out.astype(np.float64).sum(axis=0)
    loss = s[0] / (bsz * n) + s[1] / (bsz * m)
    return np.float32(loss)
